# revision 20
# baseline (speedup 1.0000x reference)
"""Qwen3.5 vision attention (S=4096, H=16x80, block-diag mask) on 8 trn2 cores.

Sharding: tensor-parallel over heads (2 heads/core). Each core computes
qkv projection for its heads, rotary, block-sparse attention, and a partial
output projection (RowParallel); the host sums the 8 partials (all-reduce)
and adds proj_b.

Device layout notes:
  - X^T [1280, 4096] (host-transposed) is the shared lhsT source for the
    QKV matmuls (natural [s, d] outputs).
  - rotary is applied in natural layout (free-dim shifts only).
  - q', k' are transposed to [d, s] via PE transpose for the score matmuls.
  - scores are computed transposed: S^T[t, s] = sum_d k[t,d] q[s,d]; softmax
    uses exp without max subtraction (scores are O(3), safe in fp32); the
    denominator comes free from a ones-column appended to V in the PV matmul.
  - normalization multiplies by reciprocal(denominator) broadcast across
    partitions (SBUF->SBUF DMA broadcast).
"""

import os
from contextlib import ExitStack

import numpy as np

S = 4096
HID = 1280
D = 80
NB = 4
BS = 1024
NHL = 2  # heads per core
NCORES = 8
SCALING = float(D) ** -0.5
NEG_THRESH = -1e8

_CACHE = {}


def _build(dt_big_name, allowed, mask_add, repeat=1, out_dt_name="float32",
           qkv_dt_name=None):
    """Build + compile the per-core bass module.

    allowed: tuple over qb of tuple of kb blocks attended to.
    mask_add: frozenset of (qb, kb) needing an additive mask tile.
    """
    import concourse.bass as bass
    import concourse.mybir as mybir
    import concourse.tile as tile
    from concourse import bacc
    from concourse.masks import make_identity

    f32 = mybir.dt.float32
    dt_big = getattr(mybir.dt, dt_big_name)
    out_dt = getattr(mybir.dt, out_dt_name)
    dt_qkv = getattr(mybir.dt, qkv_dt_name) if qkv_dt_name else dt_big
    use_mask = len(mask_add) > 0

    nc = bacc.Bacc(
        "TRN2", target_bir_lowering=False, debug=False, num_devices=NCORES
    )
    xt = nc.dram_tensor("xt", [HID, S], dt_qkv, kind="ExternalInput").ap()
    wt = nc.dram_tensor("wt", [HID, 480], dt_qkv, kind="ExternalInput").ap()
    bqkv = nc.dram_tensor("bqkv", [1, 480], f32, kind="ExternalInput").ap()
    cosd = nc.dram_tensor("cosd", [S, D], f32, kind="ExternalInput").ap()
    sind = nc.dram_tensor("sind", [S, D], f32, kind="ExternalInput").ap()
    pw = nc.dram_tensor("pw", [2, D, HID], dt_big, kind="ExternalInput").ap()
    vpad = nc.dram_tensor("vpad", [2, 17], dt_big, kind="ExternalInput").ap()
    if use_mask:
        maskt = nc.dram_tensor("maskt", [S, S], f32, kind="ExternalInput").ap()
    outp = nc.dram_tensor("outp", [HID, S], out_dt, kind="ExternalOutput").ap()

    EXP = mybir.ActivationFunctionType.Exp
    interleave = all(tuple(allowed[b]) == (b,) for b in range(NB))

    with ExitStack() as ctx:
        tc = ctx.enter_context(tile.TileContext(nc))

        # ---- constants ----
        cpool = ctx.enter_context(tc.tile_pool(name="cpool", bufs=1))
        wt_sb = cpool.tile([128, 10, 480], dt_qkv, tag="wt_sb", name="wt_sb")
        nc.sync.dma_start(out=wt_sb, in_=wt.rearrange("(kk p) c -> p kk c", p=128))
        pw_sb = cpool.tile([D, 2, HID], dt_big, tag="pw_sb", name="pw_sb")
        nc.sync.dma_start(out=pw_sb, in_=pw.rearrange("h d o -> d h o"))
        bias_bc = cpool.tile([128, 480], f32, tag="bias_bc", name="bias_bc")
        nc.sync.dma_start(out=bias_bc, in_=bqkv[0:1, :].to_broadcast((128, 480)))
        ident = cpool.tile([128, 128], f32, tag="ident", name="ident")
        make_identity(nc, ident)

        kT_sb = [
            cpool.tile([D, S], dt_big, tag=f"kT{h}_sb", name=f"kT{h}_sb")
            for h in range(NHL)
        ]

        # ---- pools ----
        xtp = ctx.enter_context(tc.tile_pool(name="xtp", bufs=11))
        trig = ctx.enter_context(tc.tile_pool(name="trig", bufs=1))
        qkp = ctx.enter_context(tc.tile_pool(name="qkp", bufs=1))
        t2p = ctx.enter_context(tc.tile_pool(name="t2p", bufs=2))
        vp = ctx.enter_context(tc.tile_pool(name="vp", bufs=1))
        qtp = ctx.enter_context(
            tc.tile_pool(name="qtp", bufs=2 if interleave else 8)
        )
        expp = ctx.enter_context(tc.tile_pool(name="expp", bufs=3))
        otp = ctx.enter_context(tc.tile_pool(name="otp", bufs=4))
        rbp = ctx.enter_context(tc.tile_pool(name="rbp", bufs=2))
        rdp = ctx.enter_context(tc.tile_pool(name="rdp", bufs=2, space="DRAM"))
        stg = ctx.enter_context(tc.tile_pool(name="stg", bufs=4))
        if use_mask:
            mtp = ctx.enter_context(tc.tile_pool(name="mtp", bufs=4))

        qkvps = ctx.enter_context(tc.tile_pool(name="qkvps", bufs=2, space="PSUM"))
        trps = ctx.enter_context(tc.tile_pool(name="trps", bufs=1, space="PSUM"))
        scps = ctx.enter_context(tc.tile_pool(name="scps", bufs=2, space="PSUM"))
        pvps = ctx.enter_context(tc.tile_pool(name="pvps", bufs=1, space="PSUM"))
        prps = ctx.enter_context(tc.tile_pool(name="prps", bufs=2, space="PSUM"))

        v_tiles = {}
        qT_tiles = {}

        def phase_qkv(b, rep=0):
            xts = []
            for k in range(10):
                xt_t = xtp.tile([128, BS], dt_qkv, tag="xt", name=f"xt_{rep}_{b}_{k}")
                nc.sync.dma_start(
                    out=xt_t, in_=xt[k * 128 : (k + 1) * 128, b * BS : (b + 1) * BS]
                )
                xts.append(xt_t)
            cos_t = trig.tile([128, 8, D], f32, tag="cos", name=f"cos_{rep}_{b}")
            nc.sync.dma_start(
                out=cos_t,
                in_=cosd[b * BS : (b + 1) * BS, :].rearrange("(c p) d -> p c d", p=128),
            )
            sin_t = trig.tile([128, 8, D], f32, tag="sin", name=f"sin_{rep}_{b}")
            nc.sync.dma_start(
                out=sin_t,
                in_=sind[b * BS : (b + 1) * BS, :].rearrange("(c p) d -> p c d", p=128),
            )

            qk_blk = qkp.tile([128, 8, 320], f32, tag="qk", name=f"qk_{rep}_{b}")
            for m in range(8):
                ps = qkvps.tile([128, 480], f32, tag="qkvps", name=f"qkvps_{rep}_{b}_{m}")
                for k in range(10):
                    nc.tensor.matmul(
                        out=ps,
                        lhsT=xts[k][:, m * 128 : (m + 1) * 128],
                        rhs=wt_sb[:, k, :],
                        start=(k == 0),
                        stop=(k == 9),
                    )
                nc.vector.tensor_add(
                    out=qk_blk[:, m, :], in0=ps[:, 0:320], in1=bias_bc[:, 0:320]
                )
                v_t = vp.tile(
                    [128, 194], dt_big, tag="v", name=f"v_{rep}_{b}_{m}",
                    bufs=16 if interleave else 32,
                )
                v3 = v_t.rearrange("p (h c) -> p h c", h=2)
                nc.vector.tensor_add(
                    out=v3[:, :, 0:D],
                    in0=ps[:, 320:480].rearrange("p (h d) -> p h d", h=2),
                    in1=bias_bc[:, 320:480].rearrange("p (h d) -> p h d", h=2),
                )
                vpad_bc = __import__("concourse.bass", fromlist=["AP"]).AP(
                    tensor=vpad.tensor, offset=0, ap=[[0, 128], [17, 2], [1, 17]]
                )
                nc.sync.dma_start(out=v3[:, :, D:97], in_=vpad_bc)
                v_tiles[(b, m)] = v_t

            # rotary (in place on qk_blk) + transpose to [d, s]
            for tau in range(2):  # 0=q, 1=k
                for h in range(NHL):
                    base = tau * 160 + h * D
                    sl = qk_blk[:, :, base : base + D]
                    t2 = t2p.tile([128, 8, D], f32, tag="t2", name=f"t2_{rep}_{b}_{tau}_{h}")
                    nc.vector.tensor_mul(
                        out=t2[:, :, 0:40],
                        in0=qk_blk[:, :, base + 40 : base + D],
                        in1=sin_t[:, :, 0:40],
                    )
                    nc.vector.tensor_mul(
                        out=t2[:, :, 40:D],
                        in0=qk_blk[:, :, base : base + 40],
                        in1=sin_t[:, :, 40:D],
                    )
                    nc.vector.tensor_mul(out=sl, in0=sl, in1=cos_t)
                    nc.vector.tensor_add(out=sl, in0=sl, in1=t2)

                    if tau == 0:
                        dst_t = qtp.tile(
                            [D, BS], dt_big, tag="qt", name=f"qT_{rep}_{b}_{h}"
                        )
                        qT_tiles[(b, h)] = dst_t
                    for g in range(2):
                        tp = trps.tile(
                            [D, 512], f32, tag="trps", name=f"tr_{rep}_{b}_{tau}_{h}_{g}"
                        )
                        for j in range(4):
                            m = g * 4 + j
                            nc.tensor.matmul(
                                out=tp[:, j * 128 : (j + 1) * 128],
                                lhsT=qk_blk[:, m, base : base + D],
                                rhs=ident,
                                is_transpose=True,
                                start=(j == 0),
                                stop=(j == 3),
                            )
                        if tau == 0:
                            dst = qT_tiles[(b, h)][:, g * 512 : (g + 1) * 512]
                        else:
                            dst = kT_sb[h][:, b * BS + g * 512 : b * BS + (g + 1) * 512]
                        nc.vector.tensor_copy(out=dst, in_=tp)

        def phase_attn(b, rep=0):
            kbs = allowed[b]
            nmm = len(kbs) * 8
            ots = []
            for h in range(NHL):
                qT_t = qT_tiles[(b, h)]
                ot_t = otp.tile([D, BS], dt_big, tag="ot", name=f"ot_{rep}_{b}_{h}")
                ots.append(ot_t)
                for sh in range(2):
                    pv = pvps.tile([97, 512], f32, tag="pv", name=f"pv_{rep}_{b}_{h}_{sh}")
                    i = 0
                    for kb in kbs:
                        for t in range(8):
                            scp = scps.tile(
                                [128, 512], f32, tag="sc",
                                name=f"sc_{rep}_{b}_{h}_{sh}_{kb}_{t}",
                            )
                            nc.tensor.matmul(
                                out=scp,
                                lhsT=kT_sb[h][
                                    :, kb * BS + t * 128 : kb * BS + (t + 1) * 128
                                ],
                                rhs=qT_t[:, sh * 512 : (sh + 1) * 512],
                                start=True,
                                stop=True,
                            )
                            if (b, kb) in mask_add:
                                mk = mtp.tile(
                                    [128, 512], f32, tag="mk",
                                    name=f"mk_{rep}_{b}_{h}_{sh}_{kb}_{t}",
                                )
                                nc.sync.dma_start(
                                    out=mk,
                                    in_=maskt[
                                        kb * BS + t * 128 : kb * BS + (t + 1) * 128,
                                        b * BS + sh * 512 : b * BS + (sh + 1) * 512,
                                    ],
                                )
                                nc.vector.tensor_add(out=scp, in0=scp, in1=mk)
                            ep = expp.tile(
                                [128, 512], dt_big, tag="exp",
                                name=f"ep_{rep}_{b}_{h}_{sh}_{kb}_{t}",
                            )
                            nc.scalar.activation(
                                out=ep, in_=scp, func=EXP, scale=SCALING
                            )
                            nc.tensor.matmul(
                                out=pv,
                                lhsT=v_tiles[(kb, t)][
                                    :, h * 97 : (h + 1) * 97
                                ],
                                rhs=ep,
                                start=(i == 0),
                                stop=(i == nmm - 1),
                            )
                            i += 1
                    rb = rbp.tile([80, 512], f32, tag="rb", name=f"rb_{rep}_{b}_{h}_{sh}")
                    nc.vector.reciprocal(out=rb[0:1, :], in_=pv[96:97, :])
                    rd = rdp.tile(
                        [1, 512], f32, tag="rd", name=f"rd_{rep}_{b}_{h}_{sh}"
                    )
                    nc.sync.dma_start(out=rd, in_=rb[0:1, :])
                    nc.sync.dma_start(
                        out=rb[0:80, :], in_=rd[0:1, :].to_broadcast((80, 512))
                    )
                    nc.vector.tensor_mul(
                        out=ot_t[:, sh * 512 : (sh + 1) * 512],
                        in0=pv[0:80, :],
                        in1=rb[0:80, :],
                    )
            # output projection (partial over this core's 160 dims)
            ot0, ot1 = ots
            for sh in range(2):
                for o in range(10):
                    pp = prps.tile(
                        [128, 512], f32, tag="pr", name=f"pr_{rep}_{b}_{sh}_{o}"
                    )
                    nc.tensor.matmul(
                        out=pp,
                        lhsT=pw_sb[:, 0, o * 128 : (o + 1) * 128],
                        rhs=ot0[:, sh * 512 : (sh + 1) * 512],
                        start=True,
                        stop=False,
                    )
                    nc.tensor.matmul(
                        out=pp,
                        lhsT=pw_sb[:, 1, o * 128 : (o + 1) * 128],
                        rhs=ot1[:, sh * 512 : (sh + 1) * 512],
                        start=False,
                        stop=True,
                    )
                    st = stg.tile(
                        [128, 512], out_dt, tag="st", name=f"st_{rep}_{b}_{sh}_{o}"
                    )
                    nc.scalar.copy(out=st, in_=pp)
                    nc.sync.dma_start(
                        out=outp[
                            o * 128 : (o + 1) * 128,
                            b * BS + sh * 512 : b * BS + (sh + 1) * 512,
                        ],
                        in_=st,
                    )

        for rep in range(repeat):
            if interleave:
                for b in range(NB):
                    phase_qkv(b, rep)
                    phase_attn(b, rep)
            else:
                for b in range(NB):
                    phase_qkv(b, rep)
                for b in range(NB):
                    phase_attn(b, rep)

    nc.compile()
    return nc


def _build_diag():
    """Block-diagonal fast path: core = (block, head-half).

    Each core computes 8 heads of one 1024-token block: QKV (bf16 matmuls),
    rotary (bf16 DVE), PE transposes of q/k to [d, s], block attention with
    exp batched in [128, 1536] PSUM-spanning ACT ops, softmax denominator
    reciprocal via ACT Ln -> Exp(scale=-1) (same table set as exp), packed
    128-deep output projection, f32 partial [1280, 1024] out.
    """
    import functools

    import concourse.bass as bass
    import concourse.mybir as mybir
    import concourse.tile as tile
    from concourse import bacc
    from concourse import hw_specs
    from concourse.masks import make_identity

    f32 = mybir.dt.float32
    bf16 = mybir.dt.bfloat16
    EXP = mybir.ActivationFunctionType.Exp
    LN = mybir.ActivationFunctionType.Ln

    # Pin Exp/Ln/Copy/Identity to the one table set containing them all, so
    # the act-table-load pass emits a single load instead of flapping
    # between exp_and_others and natural_log_exp_and_others per call.
    if not getattr(bacc, "_act_tables_pinned", False):
        orig_gat = bacc.get_activation_tables
        pinned = {
            mybir.ActivationFunctionType.Exp,
            mybir.ActivationFunctionType.Ln,
            mybir.ActivationFunctionType.Copy,
            mybir.ActivationFunctionType.Identity,
        }

        @functools.cache
        def _gat(arch):
            out = {}
            for name, fns in orig_gat(arch).items():
                if name == "natural_log_exp_and_others":
                    out[name] = set(fns)
                else:
                    out[name] = set(fns) - pinned
            return out

        bacc.get_activation_tables = _gat
        bacc._act_tables_pinned = True

    B = 1024  # block length
    NH = 8  # heads per core
    QKW = 3 * NH * D  # 1920

    nc = bacc.Bacc(
        "TRN2", target_bir_lowering=False, debug=False, num_devices=NCORES
    )
    xt = nc.dram_tensor("xt", [HID, B], bf16, kind="ExternalInput").ap()
    wt = nc.dram_tensor("wt", [HID, QKW], bf16, kind="ExternalInput").ap()
    bqkv = nc.dram_tensor("bqkv", [1, QKW], bf16, kind="ExternalInput").ap()
    cosd = nc.dram_tensor("cosd", [B, D], bf16, kind="ExternalInput").ap()
    sind = nc.dram_tensor("sind", [B, D], bf16, kind="ExternalInput").ap()
    pw = nc.dram_tensor("pw", [NH * D, HID], bf16, kind="ExternalInput").ap()
    vones = nc.dram_tensor("vones", [1, 17], bf16, kind="ExternalInput").ap()
    outp = nc.dram_tensor("outp", [HID, B], f32, kind="ExternalOutput").ap()

    with ExitStack() as ctx:
        tc = ctx.enter_context(tile.TileContext(nc))

        # ---- persistent SBUF ----
        cpool = ctx.enter_context(tc.tile_pool(name="cpool", bufs=1))
        wt_sb = cpool.tile([128, 10, QKW], bf16, tag="wt_sb")
        xt_sb = cpool.tile([128, 10, B], bf16, tag="xt_sb")
        for k in range(10):
            nc.sync.dma_start(
                out=xt_sb[:, k, :], in_=xt[k * 128 : (k + 1) * 128, :]
            )
            nc.sync.dma_start(
                out=wt_sb[:, k, :], in_=wt[k * 128 : (k + 1) * 128, :]
            )
        bias_bc = cpool.tile([128, QKW], bf16, tag="bias_bc")
        nc.sync.dma_start(out=bias_bc, in_=bqkv[0:1, :].to_broadcast((128, QKW)))
        cos_t = cpool.tile([128, 8, D], bf16, tag="cos_t")
        nc.sync.dma_start(
            out=cos_t, in_=cosd.rearrange("(c p) d -> p c d", p=128)
        )
        sin_t = cpool.tile([128, 8, D], bf16, tag="sin_t")
        nc.sync.dma_start(
            out=sin_t, in_=sind.rearrange("(c p) d -> p c d", p=128)
        )
        pw_sb = cpool.tile([128, 5, HID], bf16, tag="pw_sb")
        nc.sync.dma_start(out=pw_sb, in_=pw.rearrange("(j p) o -> p j o", p=128))
        ident = cpool.tile([128, 128], bf16, tag="ident")
        make_identity(nc, ident)

        qk_blk = cpool.tile([128, 8, 2 * NH * D], bf16, tag="qk_blk")
        v_sb = cpool.tile([128, 8, NH, 97], bf16, tag="v_sb")
        vones_bc = bass.AP(
            tensor=vones.tensor, offset=0, ap=[[0, 128], [0, 8 * NH], [1, 17]]
        )
        nc.sync.dma_start(
            out=v_sb.rearrange("p m h c -> p (m h) c")[:, :, D:97], in_=vones_bc
        )
        kT_sb = cpool.tile([D, NH, B], bf16, tag="kT_sb")
        qT_sb = cpool.tile([D, NH, B], bf16, tag="qT_sb")
        ao = cpool.tile([128, 5, B], bf16, tag="ao")
        ot_sb = [
            cpool.tile([D, B], bf16, tag=f"ot{h}", name=f"ot{h}")
            for h in range(NH)
        ]

        # ---- pools ----
        bps = ctx.enter_context(tc.tile_pool(name="bps", bufs=2, space="PSUM"))
        sps = ctx.enter_context(tc.tile_pool(name="sps", bufs=4, space="PSUM"))
        t2p = ctx.enter_context(tc.tile_pool(name="t2p", bufs=4))
        expp = ctx.enter_context(tc.tile_pool(name="expp", bufs=3))
        lnp = ctx.enter_context(tc.tile_pool(name="lnp", bufs=4))
        rep = ctx.enter_context(tc.tile_pool(name="rep", bufs=4))
        rbp = ctx.enter_context(tc.tile_pool(name="rbp", bufs=4))
        rdp = ctx.enter_context(tc.tile_pool(name="rdp", bufs=4, space="DRAM"))
        stg = ctx.enter_context(tc.tile_pool(name="stg", bufs=4))

        # ---- phase 1: QKV + rotary ----
        def rotary_half(g):
            # split (tau, h) combos between DVE and GpSimd so the next
            # m-tile's bias-adds are not queued behind rotary on DVE
            ms = slice(g * 4, (g + 1) * 4)
            for tau in range(2):
                for h in range(NH):
                    eng = nc.gpsimd if (tau * NH + h) % 3 == 2 else nc.vector
                    base = tau * NH * D + h * D
                    sl = qk_blk[:, ms, base : base + D]
                    t2 = t2p.tile([128, 4, D], bf16, tag="t2", name=f"t2_{g}_{tau}_{h}")
                    eng.tensor_mul(
                        out=t2[:, :, 0:40],
                        in0=qk_blk[:, ms, base + 40 : base + D],
                        in1=sin_t[:, ms, 0:40],
                    )
                    eng.tensor_mul(
                        out=t2[:, :, 40:D],
                        in0=qk_blk[:, ms, base : base + 40],
                        in1=sin_t[:, ms, 40:D],
                    )
                    eng.tensor_mul(out=sl, in0=sl, in1=cos_t[:, ms, :])
                    eng.tensor_add(out=sl, in0=sl, in1=t2)

        for m in range(8):
            ps_a = bps.tile([128, 1024], f32, tag="big", name=f"qkva_{m}")
            ps_b = bps.tile([128, 1024], f32, tag="big", name=f"qkvb_{m}")
            for j in range(4):
                ps = ps_a if j < 2 else ps_b
                dst = ps[:, (j % 2) * 512 : (j % 2) * 512 + 480]
                for k in range(10):
                    nc.tensor.matmul(
                        out=dst,
                        lhsT=xt_sb[:, k, m * 128 : (m + 1) * 128],
                        rhs=wt_sb[:, k, j * 480 : (j + 1) * 480],
                        start=(k == 0),
                        stop=(k == 9),
                    )
            # bias adds: q,k -> qk_blk; v -> v_sb
            nc.vector.tensor_add(
                out=qk_blk[:, m, 0:480], in0=ps_a[:, 0:480], in1=bias_bc[:, 0:480]
            )
            nc.vector.tensor_add(
                out=qk_blk[:, m, 480:960],
                in0=ps_a[:, 512:992],
                in1=bias_bc[:, 480:960],
            )
            nc.vector.tensor_add(
                out=qk_blk[:, m, 960:1280],
                in0=ps_b[:, 0:320],
                in1=bias_bc[:, 960:1280],
            )
            nc.vector.tensor_add(
                out=v_sb[:, m, 0:2, 0:D],
                in0=ps_b[:, 320:480].rearrange("p (h d) -> p h d", h=2),
                in1=bias_bc[:, 1280:1440].rearrange("p (h d) -> p h d", h=2),
            )
            nc.vector.tensor_add(
                out=v_sb[:, m, 2:NH, 0:D],
                in0=ps_b[:, 512:992].rearrange("p (h d) -> p h d", h=6),
                in1=bias_bc[:, 1440:1920].rearrange("p (h d) -> p h d", h=6),
            )
            if m == 3:
                rotary_half(0)
            elif m == 7:
                rotary_half(1)

        # ---- phases 2+3 interleaved per head: transposes then attention ----
        def transpose_head(h):
            for tau in (1, 0):  # k first so scores unblock sooner
                for g in range(2):
                    base = tau * NH * D + h * D
                    tp = sps.tile(
                        [97, 512], bf16, tag="small", name=f"tr_{g}_{tau}_{h}"
                    )
                    for j in range(4):
                        m = g * 4 + j
                        nc.tensor.matmul(
                            out=tp[0:D, j * 128 : (j + 1) * 128],
                            lhsT=qk_blk[:, m, base : base + D],
                            rhs=ident,
                            is_transpose=True,
                            start=(j == 0),
                            stop=(j == 3),
                        )
                    dst = (qT_sb if tau == 0 else kT_sb)[
                        :, h, g * 512 : (g + 1) * 512
                    ]
                    if (tau + g) % 2 == 0:
                        nc.scalar.copy(out=dst, in_=tp[0:D, :])
                    else:
                        nc.vector.tensor_copy(out=dst, in_=tp[0:D, :])

        for h in range(NH):
            transpose_head(h)
            for sh in range(2):
                qs = qT_sb[:, h, sh * 512 : (sh + 1) * 512]
                pv = sps.tile([97, 512], f32, tag="small", name=f"pv_{h}_{sh}")
                eps = []
                for grp, ts in enumerate(((0, 1), (2, 3), (4, 5), (6, 7))):
                    sc = bps.tile([128, 1024], f32, tag="big", name=f"sc_{h}_{sh}_{grp}")
                    for i, t in enumerate(ts):
                        nc.tensor.matmul(
                            out=sc[:, i * 512 : (i + 1) * 512],
                            lhsT=kT_sb[:, h, t * 128 : (t + 1) * 128],
                            rhs=qs,
                            start=True,
                            stop=True,
                        )
                    w = len(ts) * 512
                    ep = expp.tile([128, 1024], bf16, tag="exp", name=f"ep_{h}_{sh}_{grp}")
                    nc.scalar.activation(
                        out=ep[:, 0:w], in_=sc[:, 0:w], func=EXP, scale=SCALING
                    )
                    eps.append(ep)
                    for i, t in enumerate(ts):
                        nc.tensor.matmul(
                            out=pv,
                            lhsT=v_sb[:, t, h, :],
                            rhs=ep[:, i * 512 : (i + 1) * 512],
                            start=(t == 0),
                            stop=(t == 7),
                        )
                # softmax normalization: 1/den via Ln -> Exp(-x)
                ln_t = lnp.tile([1, 512], f32, tag="ln")
                nc.scalar.activation(out=ln_t, in_=pv[96:97, :], func=LN)
                re_t = rep.tile([1, 512], bf16, tag="re")
                nc.scalar.activation(out=re_t, in_=ln_t, func=EXP, scale=-1.0)
                rd_t = rdp.tile([1, 512], bf16, tag="rd")
                nc.sync.dma_start(out=rd_t, in_=re_t)
                rb_t = rbp.tile([D, 512], bf16, tag="rb")
                nc.sync.dma_start(out=rb_t, in_=rd_t[0:1, :].to_broadcast((D, 512)))
                nc.vector.tensor_mul(
                    out=ot_sb[h][:, sh * 512 : (sh + 1) * 512],
                    in0=pv[0:D, :],
                    in1=rb_t,
                )
            # repack this head into the 128-deep contraction layout
            r0 = h * D
            a = r0
            while a < r0 + D:
                j, p0 = a // 128, a % 128
                b_ = min(r0 + D, (j + 1) * 128)
                nc.sync.dma_start(
                    out=ao[p0 : p0 + (b_ - a), j, :],
                    in_=ot_sb[h][a - r0 : b_ - r0, :],
                )
                a = b_

        # ---- phase 5: output projection ----
        for sh in range(2):
            for o in range(10):
                pp = bps.tile([128, 1024], f32, tag="big", name=f"pr_{sh}_{o}")
                for j in range(5):
                    nc.tensor.matmul(
                        out=pp[:, 0:512],
                        lhsT=pw_sb[:, j, o * 128 : (o + 1) * 128],
                        rhs=ao[:, j, sh * 512 : (sh + 1) * 512],
                        start=(j == 0),
                        stop=(j == 4),
                    )
                st = stg.tile([128, 512], f32, tag="st")
                if (sh * 10 + o) % 2 == 0:
                    nc.scalar.copy(out=st, in_=pp[:, 0:512])
                else:
                    nc.vector.tensor_copy(out=st, in_=pp[:, 0:512])
                nc.sync.dma_start(
                    out=outp[
                        o * 128 : (o + 1) * 128, sh * 512 : (sh + 1) * 512
                    ],
                    in_=st,
                )

    nc.compile()
    return nc


def _analyze_mask(mask):
    m = np.asarray(mask).reshape(S, S)
    allowed = []
    mask_add = set()
    for qb in range(NB):
        row = []
        for kb in range(NB):
            t = m[qb * BS : (qb + 1) * BS, kb * BS : (kb + 1) * BS]
            if np.all(t <= NEG_THRESH):
                continue
            row.append(kb)
            if not np.all(t == 0.0):
                mask_add.add((qb, kb))
        if not row:
            raise NotImplementedError("fully masked query block")
        allowed.append(tuple(row))
    return tuple(allowed), frozenset(mask_add)


def _kernel_diag(X, cos, sin, qkv_w, qkv_b, proj_w, proj_b, trace):
    import ml_dtypes

    from concourse import bass_utils

    bf = ml_dtypes.bfloat16
    if "diag" not in _CACHE:
        _CACHE["diag"] = _build_diag()
    nc = _CACHE["diag"]

    B = 1024
    XT = np.ascontiguousarray(X.T)
    cos = np.asarray(cos, dtype=np.float32)
    sin = np.asarray(sin, dtype=np.float32)
    sinh = np.concatenate([-sin[:, : D // 2], sin[:, D // 2 :]], axis=1)
    qkv_w = np.asarray(qkv_w, dtype=np.float32)
    qkv_b = np.asarray(qkv_b, dtype=np.float32)
    proj_w = np.asarray(proj_w, dtype=np.float32)
    proj_b = np.asarray(proj_b, dtype=np.float32)
    vones = np.concatenate([np.zeros(16, bf), np.ones(1, bf)])[None, :]

    in_maps = []
    for c in range(NCORES):
        b, half = c // 2, c % 2
        j0 = half * 8 * D
        hs = slice(j0, j0 + 8 * D)
        Wc = np.concatenate(
            [qkv_w[0:HID][hs], qkv_w[HID : 2 * HID][hs], qkv_w[2 * HID :][hs]],
            axis=0,
        )
        bc = np.concatenate(
            [qkv_b[0:HID][hs], qkv_b[HID : 2 * HID][hs], qkv_b[2 * HID :][hs]]
        )
        in_maps.append(
            {
                "xt": np.ascontiguousarray(XT[:, b * B : (b + 1) * B]).astype(bf),
                "wt": np.ascontiguousarray(Wc.T).astype(bf),
                "bqkv": np.ascontiguousarray(bc[None, :]).astype(bf),
                "cosd": np.ascontiguousarray(cos[b * B : (b + 1) * B]).astype(bf),
                "sind": np.ascontiguousarray(sinh[b * B : (b + 1) * B]).astype(bf),
                "pw": np.ascontiguousarray(proj_w[:, hs].T).astype(bf),
                "vones": vones,
            }
        )

    res = bass_utils.run_bass_kernel_spmd(
        nc, in_maps, core_ids=list(range(NCORES)), trace=trace
    )
    global LAST_RESULT
    LAST_RESULT = res

    out = np.empty((S, HID), dtype=np.float32)
    for b in range(NB):
        acc = res.results[2 * b]["outp"].astype(np.float64) + res.results[
            2 * b + 1
        ]["outp"].astype(np.float64)
        out[b * B : (b + 1) * B] = (
            acc.T + proj_b.astype(np.float64)[None, :]
        ).astype(np.float32)
    return out


def kernel(
    hidden_states, attention_mask, cos, sin, qkv_w, qkv_b, proj_w, proj_b
):
    from concourse import bass_utils

    dt_big = os.environ.get("KERNEL_DT", "float32r")
    out_dt = os.environ.get("KERNEL_OUT_DT", "float32")
    qkv_dt = os.environ.get("KERNEL_QKV_DT", "") or None
    trace = bool(int(os.environ.get("KERNEL_TRACE", "0")))

    X = np.ascontiguousarray(np.asarray(hidden_states, dtype=np.float32))
    allowed, mask_add = _analyze_mask(attention_mask)

    use_diag = (
        allowed == tuple((b,) for b in range(NB))
        and not mask_add
        and not int(os.environ.get("KERNEL_LEGACY", "0"))
    )
    if use_diag:
        return _kernel_diag(X, cos, sin, qkv_w, qkv_b, proj_w, proj_b, trace)

    key = (dt_big, out_dt, qkv_dt, allowed, mask_add)
    if key not in _CACHE:
        _CACHE[key] = _build(
            dt_big, allowed, mask_add, out_dt_name=out_dt, qkv_dt_name=qkv_dt
        )
    nc = _CACHE[key]

    XT = np.ascontiguousarray(X.T)
    cos = np.ascontiguousarray(np.asarray(cos, dtype=np.float32))
    sin = np.asarray(sin, dtype=np.float32)
    sinh = np.ascontiguousarray(
        np.concatenate([-sin[:, : D // 2], sin[:, D // 2 :]], axis=1)
    )
    qkv_w = np.asarray(qkv_w, dtype=np.float32)
    qkv_b = np.asarray(qkv_b, dtype=np.float32)
    proj_w = np.asarray(proj_w, dtype=np.float32)
    proj_b = np.asarray(proj_b, dtype=np.float32)

    in_maps = []
    for c in range(NCORES):
        j0 = c * NHL * D
        sl = slice(j0, j0 + NHL * D)
        Wc = np.concatenate(
            [qkv_w[sl], qkv_w[HID:][sl], qkv_w[2 * HID :][sl]], axis=0
        )
        np_qkv = np.float32
        if qkv_dt == "bfloat16":
            import ml_dtypes

            np_qkv = ml_dtypes.bfloat16
        m = {
            "xt": XT.astype(np_qkv),
            "vpad": np.ascontiguousarray(
                np.tile(
                    np.concatenate(
                        [np.zeros(16, np.float32), np.ones(1, np.float32)]
                    ),
                    (2, 1),
                )
            ),
            "wt": np.ascontiguousarray(Wc.T).astype(np_qkv),
            "bqkv": np.ascontiguousarray(
                np.concatenate([qkv_b[sl], qkv_b[HID:][sl], qkv_b[2 * HID :][sl]])[
                    None, :
                ]
            ),
            "cosd": cos,
            "sind": sinh,
            "pw": np.ascontiguousarray(
                np.stack(
                    [
                        proj_w[:, j0 : j0 + D].T,
                        proj_w[:, j0 + D : j0 + 2 * D].T,
                    ]
                )
            ),
        }
        if mask_add:
            m["maskt"] = np.ascontiguousarray(
                (np.asarray(attention_mask).reshape(S, S).T / SCALING).astype(
                    np.float32
                )
            )
        in_maps.append(m)

    res = bass_utils.run_bass_kernel_spmd(
        nc, in_maps, core_ids=list(range(NCORES)), trace=trace
    )
    global LAST_RESULT
    LAST_RESULT = res

    acc = np.zeros((HID, S), dtype=np.float64)
    for c in range(NCORES):
        acc += res.results[c]["outp"]
    out = acc.T + proj_b.astype(np.float64)[None, :]
    return out.astype(np.float32)


LAST_RESULT = None



# revision 26
# speedup vs baseline: 1.0205x; 1.0205x over previous
"""Qwen3.5 vision attention (S=4096, H=16x80, block-diag mask) on 8 trn2 cores.

Sharding: tensor-parallel over heads (2 heads/core). Each core computes
qkv projection for its heads, rotary, block-sparse attention, and a partial
output projection (RowParallel); the host sums the 8 partials (all-reduce)
and adds proj_b.

Device layout notes:
  - X^T [1280, 4096] (host-transposed) is the shared lhsT source for the
    QKV matmuls (natural [s, d] outputs).
  - rotary is applied in natural layout (free-dim shifts only).
  - q', k' are transposed to [d, s] via PE transpose for the score matmuls.
  - scores are computed transposed: S^T[t, s] = sum_d k[t,d] q[s,d]; softmax
    uses exp without max subtraction (scores are O(3), safe in fp32); the
    denominator comes free from a ones-column appended to V in the PV matmul.
  - normalization multiplies by reciprocal(denominator) broadcast across
    partitions (SBUF->SBUF DMA broadcast).
"""

import os
from contextlib import ExitStack

import numpy as np

S = 4096
HID = 1280
D = 80
NB = 4
BS = 1024
NHL = 2  # heads per core
NCORES = 8
SCALING = float(D) ** -0.5
NEG_THRESH = -1e8

_CACHE = {}


def _build(dt_big_name, allowed, mask_add, repeat=1, out_dt_name="float32",
           qkv_dt_name=None):
    """Build + compile the per-core bass module.

    allowed: tuple over qb of tuple of kb blocks attended to.
    mask_add: frozenset of (qb, kb) needing an additive mask tile.
    """
    import concourse.bass as bass
    import concourse.mybir as mybir
    import concourse.tile as tile
    from concourse import bacc
    from concourse.masks import make_identity

    f32 = mybir.dt.float32
    dt_big = getattr(mybir.dt, dt_big_name)
    out_dt = getattr(mybir.dt, out_dt_name)
    dt_qkv = getattr(mybir.dt, qkv_dt_name) if qkv_dt_name else dt_big
    use_mask = len(mask_add) > 0

    nc = bacc.Bacc(
        "TRN2", target_bir_lowering=False, debug=False, num_devices=NCORES
    )
    xt = nc.dram_tensor("xt", [HID, S], dt_qkv, kind="ExternalInput").ap()
    wt = nc.dram_tensor("wt", [HID, 480], dt_qkv, kind="ExternalInput").ap()
    bqkv = nc.dram_tensor("bqkv", [1, 480], f32, kind="ExternalInput").ap()
    cosd = nc.dram_tensor("cosd", [S, D], f32, kind="ExternalInput").ap()
    sind = nc.dram_tensor("sind", [S, D], f32, kind="ExternalInput").ap()
    pw = nc.dram_tensor("pw", [2, D, HID], dt_big, kind="ExternalInput").ap()
    vpad = nc.dram_tensor("vpad", [2, 17], dt_big, kind="ExternalInput").ap()
    if use_mask:
        maskt = nc.dram_tensor("maskt", [S, S], f32, kind="ExternalInput").ap()
    outp = nc.dram_tensor("outp", [HID, S], out_dt, kind="ExternalOutput").ap()

    EXP = mybir.ActivationFunctionType.Exp
    interleave = all(tuple(allowed[b]) == (b,) for b in range(NB))

    with ExitStack() as ctx:
        tc = ctx.enter_context(tile.TileContext(nc))

        # ---- constants ----
        cpool = ctx.enter_context(tc.tile_pool(name="cpool", bufs=1))
        wt_sb = cpool.tile([128, 10, 480], dt_qkv, tag="wt_sb", name="wt_sb")
        nc.sync.dma_start(out=wt_sb, in_=wt.rearrange("(kk p) c -> p kk c", p=128))
        pw_sb = cpool.tile([D, 2, HID], dt_big, tag="pw_sb", name="pw_sb")
        nc.sync.dma_start(out=pw_sb, in_=pw.rearrange("h d o -> d h o"))
        bias_bc = cpool.tile([128, 480], f32, tag="bias_bc", name="bias_bc")
        nc.sync.dma_start(out=bias_bc, in_=bqkv[0:1, :].to_broadcast((128, 480)))
        ident = cpool.tile([128, 128], f32, tag="ident", name="ident")
        make_identity(nc, ident)

        kT_sb = [
            cpool.tile([D, S], dt_big, tag=f"kT{h}_sb", name=f"kT{h}_sb")
            for h in range(NHL)
        ]

        # ---- pools ----
        xtp = ctx.enter_context(tc.tile_pool(name="xtp", bufs=11))
        trig = ctx.enter_context(tc.tile_pool(name="trig", bufs=1))
        qkp = ctx.enter_context(tc.tile_pool(name="qkp", bufs=1))
        t2p = ctx.enter_context(tc.tile_pool(name="t2p", bufs=2))
        vp = ctx.enter_context(tc.tile_pool(name="vp", bufs=1))
        qtp = ctx.enter_context(
            tc.tile_pool(name="qtp", bufs=2 if interleave else 8)
        )
        expp = ctx.enter_context(tc.tile_pool(name="expp", bufs=3))
        otp = ctx.enter_context(tc.tile_pool(name="otp", bufs=4))
        rbp = ctx.enter_context(tc.tile_pool(name="rbp", bufs=2))
        rdp = ctx.enter_context(tc.tile_pool(name="rdp", bufs=2, space="DRAM"))
        stg = ctx.enter_context(tc.tile_pool(name="stg", bufs=4))
        if use_mask:
            mtp = ctx.enter_context(tc.tile_pool(name="mtp", bufs=4))

        qkvps = ctx.enter_context(tc.tile_pool(name="qkvps", bufs=2, space="PSUM"))
        trps = ctx.enter_context(tc.tile_pool(name="trps", bufs=1, space="PSUM"))
        scps = ctx.enter_context(tc.tile_pool(name="scps", bufs=2, space="PSUM"))
        pvps = ctx.enter_context(tc.tile_pool(name="pvps", bufs=1, space="PSUM"))
        prps = ctx.enter_context(tc.tile_pool(name="prps", bufs=2, space="PSUM"))

        v_tiles = {}
        qT_tiles = {}

        def phase_qkv(b, rep=0):
            xts = []
            for k in range(10):
                xt_t = xtp.tile([128, BS], dt_qkv, tag="xt", name=f"xt_{rep}_{b}_{k}")
                nc.sync.dma_start(
                    out=xt_t, in_=xt[k * 128 : (k + 1) * 128, b * BS : (b + 1) * BS]
                )
                xts.append(xt_t)
            cos_t = trig.tile([128, 8, D], f32, tag="cos", name=f"cos_{rep}_{b}")
            nc.sync.dma_start(
                out=cos_t,
                in_=cosd[b * BS : (b + 1) * BS, :].rearrange("(c p) d -> p c d", p=128),
            )
            sin_t = trig.tile([128, 8, D], f32, tag="sin", name=f"sin_{rep}_{b}")
            nc.sync.dma_start(
                out=sin_t,
                in_=sind[b * BS : (b + 1) * BS, :].rearrange("(c p) d -> p c d", p=128),
            )

            qk_blk = qkp.tile([128, 8, 320], f32, tag="qk", name=f"qk_{rep}_{b}")
            for m in range(8):
                ps = qkvps.tile([128, 480], f32, tag="qkvps", name=f"qkvps_{rep}_{b}_{m}")
                for k in range(10):
                    nc.tensor.matmul(
                        out=ps,
                        lhsT=xts[k][:, m * 128 : (m + 1) * 128],
                        rhs=wt_sb[:, k, :],
                        start=(k == 0),
                        stop=(k == 9),
                    )
                nc.vector.tensor_add(
                    out=qk_blk[:, m, :], in0=ps[:, 0:320], in1=bias_bc[:, 0:320]
                )
                v_t = vp.tile(
                    [128, 194], dt_big, tag="v", name=f"v_{rep}_{b}_{m}",
                    bufs=16 if interleave else 32,
                )
                v3 = v_t.rearrange("p (h c) -> p h c", h=2)
                nc.vector.tensor_add(
                    out=v3[:, :, 0:D],
                    in0=ps[:, 320:480].rearrange("p (h d) -> p h d", h=2),
                    in1=bias_bc[:, 320:480].rearrange("p (h d) -> p h d", h=2),
                )
                vpad_bc = __import__("concourse.bass", fromlist=["AP"]).AP(
                    tensor=vpad.tensor, offset=0, ap=[[0, 128], [17, 2], [1, 17]]
                )
                nc.sync.dma_start(out=v3[:, :, D:97], in_=vpad_bc)
                v_tiles[(b, m)] = v_t

            # rotary (in place on qk_blk) + transpose to [d, s]
            for tau in range(2):  # 0=q, 1=k
                for h in range(NHL):
                    base = tau * 160 + h * D
                    sl = qk_blk[:, :, base : base + D]
                    t2 = t2p.tile([128, 8, D], f32, tag="t2", name=f"t2_{rep}_{b}_{tau}_{h}")
                    nc.vector.tensor_mul(
                        out=t2[:, :, 0:40],
                        in0=qk_blk[:, :, base + 40 : base + D],
                        in1=sin_t[:, :, 0:40],
                    )
                    nc.vector.tensor_mul(
                        out=t2[:, :, 40:D],
                        in0=qk_blk[:, :, base : base + 40],
                        in1=sin_t[:, :, 40:D],
                    )
                    nc.vector.tensor_mul(out=sl, in0=sl, in1=cos_t)
                    nc.vector.tensor_add(out=sl, in0=sl, in1=t2)

                    if tau == 0:
                        dst_t = qtp.tile(
                            [D, BS], dt_big, tag="qt", name=f"qT_{rep}_{b}_{h}"
                        )
                        qT_tiles[(b, h)] = dst_t
                    for g in range(2):
                        tp = trps.tile(
                            [D, 512], f32, tag="trps", name=f"tr_{rep}_{b}_{tau}_{h}_{g}"
                        )
                        for j in range(4):
                            m = g * 4 + j
                            nc.tensor.matmul(
                                out=tp[:, j * 128 : (j + 1) * 128],
                                lhsT=qk_blk[:, m, base : base + D],
                                rhs=ident,
                                is_transpose=True,
                                start=(j == 0),
                                stop=(j == 3),
                            )
                        if tau == 0:
                            dst = qT_tiles[(b, h)][:, g * 512 : (g + 1) * 512]
                        else:
                            dst = kT_sb[h][:, b * BS + g * 512 : b * BS + (g + 1) * 512]
                        nc.vector.tensor_copy(out=dst, in_=tp)

        def phase_attn(b, rep=0):
            kbs = allowed[b]
            nmm = len(kbs) * 8
            ots = []
            for h in range(NHL):
                qT_t = qT_tiles[(b, h)]
                ot_t = otp.tile([D, BS], dt_big, tag="ot", name=f"ot_{rep}_{b}_{h}")
                ots.append(ot_t)
                for sh in range(2):
                    pv = pvps.tile([97, 512], f32, tag="pv", name=f"pv_{rep}_{b}_{h}_{sh}")
                    i = 0
                    for kb in kbs:
                        for t in range(8):
                            scp = scps.tile(
                                [128, 512], f32, tag="sc",
                                name=f"sc_{rep}_{b}_{h}_{sh}_{kb}_{t}",
                            )
                            nc.tensor.matmul(
                                out=scp,
                                lhsT=kT_sb[h][
                                    :, kb * BS + t * 128 : kb * BS + (t + 1) * 128
                                ],
                                rhs=qT_t[:, sh * 512 : (sh + 1) * 512],
                                start=True,
                                stop=True,
                            )
                            if (b, kb) in mask_add:
                                mk = mtp.tile(
                                    [128, 512], f32, tag="mk",
                                    name=f"mk_{rep}_{b}_{h}_{sh}_{kb}_{t}",
                                )
                                nc.sync.dma_start(
                                    out=mk,
                                    in_=maskt[
                                        kb * BS + t * 128 : kb * BS + (t + 1) * 128,
                                        b * BS + sh * 512 : b * BS + (sh + 1) * 512,
                                    ],
                                )
                                nc.vector.tensor_add(out=scp, in0=scp, in1=mk)
                            ep = expp.tile(
                                [128, 512], dt_big, tag="exp",
                                name=f"ep_{rep}_{b}_{h}_{sh}_{kb}_{t}",
                            )
                            nc.scalar.activation(
                                out=ep, in_=scp, func=EXP, scale=SCALING
                            )
                            nc.tensor.matmul(
                                out=pv,
                                lhsT=v_tiles[(kb, t)][
                                    :, h * 97 : (h + 1) * 97
                                ],
                                rhs=ep,
                                start=(i == 0),
                                stop=(i == nmm - 1),
                            )
                            i += 1
                    rb = rbp.tile([80, 512], f32, tag="rb", name=f"rb_{rep}_{b}_{h}_{sh}")
                    nc.vector.reciprocal(out=rb[0:1, :], in_=pv[96:97, :])
                    rd = rdp.tile(
                        [1, 512], f32, tag="rd", name=f"rd_{rep}_{b}_{h}_{sh}"
                    )
                    nc.sync.dma_start(out=rd, in_=rb[0:1, :])
                    nc.sync.dma_start(
                        out=rb[0:80, :], in_=rd[0:1, :].to_broadcast((80, 512))
                    )
                    nc.vector.tensor_mul(
                        out=ot_t[:, sh * 512 : (sh + 1) * 512],
                        in0=pv[0:80, :],
                        in1=rb[0:80, :],
                    )
            # output projection (partial over this core's 160 dims)
            ot0, ot1 = ots
            for sh in range(2):
                for o in range(10):
                    pp = prps.tile(
                        [128, 512], f32, tag="pr", name=f"pr_{rep}_{b}_{sh}_{o}"
                    )
                    nc.tensor.matmul(
                        out=pp,
                        lhsT=pw_sb[:, 0, o * 128 : (o + 1) * 128],
                        rhs=ot0[:, sh * 512 : (sh + 1) * 512],
                        start=True,
                        stop=False,
                    )
                    nc.tensor.matmul(
                        out=pp,
                        lhsT=pw_sb[:, 1, o * 128 : (o + 1) * 128],
                        rhs=ot1[:, sh * 512 : (sh + 1) * 512],
                        start=False,
                        stop=True,
                    )
                    st = stg.tile(
                        [128, 512], out_dt, tag="st", name=f"st_{rep}_{b}_{sh}_{o}"
                    )
                    nc.scalar.copy(out=st, in_=pp)
                    nc.sync.dma_start(
                        out=outp[
                            o * 128 : (o + 1) * 128,
                            b * BS + sh * 512 : b * BS + (sh + 1) * 512,
                        ],
                        in_=st,
                    )

        for rep in range(repeat):
            if interleave:
                for b in range(NB):
                    phase_qkv(b, rep)
                    phase_attn(b, rep)
            else:
                for b in range(NB):
                    phase_qkv(b, rep)
                for b in range(NB):
                    phase_attn(b, rep)

    nc.compile()
    return nc


def _build_diag(use_div=False):
    """Block-diagonal fast path: core = (block, head-half).

    Each core computes 8 heads of one 1024-token block: QKV (bf16 matmuls),
    rotary (bf16 DVE), PE transposes of q/k to [d, s], block attention with
    exp batched in [128, 1536] PSUM-spanning ACT ops, softmax denominator
    reciprocal via ACT Ln -> Exp(scale=-1) (same table set as exp), packed
    128-deep output projection, f32 partial [1280, 1024] out.
    """
    import functools

    import concourse.bass as bass
    import concourse.mybir as mybir
    import concourse.tile as tile
    from concourse import bacc
    from concourse import hw_specs
    from concourse.masks import make_identity

    f32 = mybir.dt.float32
    bf16 = mybir.dt.bfloat16
    EXP = mybir.ActivationFunctionType.Exp
    LN = mybir.ActivationFunctionType.Ln

    # Pin Exp/Ln/Copy/Identity to the one table set containing them all, so
    # the act-table-load pass emits a single load instead of flapping
    # between exp_and_others and natural_log_exp_and_others per call.
    if not getattr(bacc, "_act_tables_pinned", False):
        orig_gat = bacc.get_activation_tables
        pinned = {
            mybir.ActivationFunctionType.Exp,
            mybir.ActivationFunctionType.Ln,
            mybir.ActivationFunctionType.Copy,
            mybir.ActivationFunctionType.Identity,
        }

        @functools.cache
        def _gat(arch):
            out = {}
            for name, fns in orig_gat(arch).items():
                if name == "natural_log_exp_and_others":
                    out[name] = set(fns)
                else:
                    out[name] = set(fns) - pinned
            return out

        bacc.get_activation_tables = _gat
        bacc._act_tables_pinned = True

    B = 1024  # block length
    NH = 8  # heads per core
    QKW = 3 * NH * D  # 1920

    nc = bacc.Bacc(
        "TRN2", target_bir_lowering=False, debug=False, num_devices=NCORES
    )
    xt = nc.dram_tensor("xt", [HID, B], bf16, kind="ExternalInput").ap()
    wt = nc.dram_tensor("wt", [HID, QKW], bf16, kind="ExternalInput").ap()
    bqkv = nc.dram_tensor("bqkv", [1, QKW], bf16, kind="ExternalInput").ap()
    cosd = nc.dram_tensor("cosd", [B, D], bf16, kind="ExternalInput").ap()
    sind = nc.dram_tensor("sind", [B, D], bf16, kind="ExternalInput").ap()
    pw = nc.dram_tensor("pw", [NH * D, HID], bf16, kind="ExternalInput").ap()
    vones = nc.dram_tensor("vones", [1, 17], bf16, kind="ExternalInput").ap()
    outp = nc.dram_tensor("outp", [HID, B], f32, kind="ExternalOutput").ap()

    with ExitStack() as ctx:
        tc = ctx.enter_context(tile.TileContext(nc))

        # ---- persistent SBUF ----
        cpool = ctx.enter_context(tc.tile_pool(name="cpool", bufs=1))
        wt_sb = cpool.tile([128, 10, QKW], bf16, tag="wt_sb")
        xt_sb = cpool.tile([128, 10, B], bf16, tag="xt_sb")
        for k in range(10):
            nc.sync.dma_start(
                out=xt_sb[:, k, :], in_=xt[k * 128 : (k + 1) * 128, :]
            )
            nc.sync.dma_start(
                out=wt_sb[:, k, :], in_=wt[k * 128 : (k + 1) * 128, :]
            )
        bias_bc = cpool.tile([128, QKW], bf16, tag="bias_bc")
        nc.sync.dma_start(out=bias_bc, in_=bqkv[0:1, :].to_broadcast((128, QKW)))
        cos_t = cpool.tile([128, 8, D], bf16, tag="cos_t")
        nc.sync.dma_start(
            out=cos_t, in_=cosd.rearrange("(c p) d -> p c d", p=128)
        )
        sin_t = cpool.tile([128, 8, D], bf16, tag="sin_t")
        nc.sync.dma_start(
            out=sin_t, in_=sind.rearrange("(c p) d -> p c d", p=128)
        )
        pw_sb = cpool.tile([128, 5, HID], bf16, tag="pw_sb")
        nc.sync.dma_start(out=pw_sb, in_=pw.rearrange("(j p) o -> p j o", p=128))
        ident = cpool.tile([128, 128], bf16, tag="ident")
        make_identity(nc, ident)

        qk_blk = cpool.tile([128, 8, 2 * NH * D], bf16, tag="qk_blk")
        v_sb = cpool.tile([128, 8, NH, 97], bf16, tag="v_sb")
        vones_bc = bass.AP(
            tensor=vones.tensor, offset=0, ap=[[0, 128], [0, 8 * NH], [1, 17]]
        )
        nc.sync.dma_start(
            out=v_sb.rearrange("p m h c -> p (m h) c")[:, :, D:97], in_=vones_bc
        )
        kT_sb = cpool.tile([D, NH, B], bf16, tag="kT_sb")
        qT_sb = cpool.tile([D, NH, B], bf16, tag="qT_sb")
        ao = cpool.tile([128, 5, B], bf16, tag="ao")
        ot_sb = [
            cpool.tile([D, B], bf16, tag=f"ot{h}", name=f"ot{h}")
            for h in range(NH)
        ]

        # ---- pools ----
        bps = ctx.enter_context(tc.tile_pool(name="bps", bufs=2, space="PSUM"))
        sps = ctx.enter_context(tc.tile_pool(name="sps", bufs=4, space="PSUM"))
        t2p = ctx.enter_context(tc.tile_pool(name="t2p", bufs=4))
        expp = ctx.enter_context(tc.tile_pool(name="expp", bufs=3))
        lnp = ctx.enter_context(tc.tile_pool(name="lnp", bufs=4))
        rep = ctx.enter_context(tc.tile_pool(name="rep", bufs=4))
        rbp = ctx.enter_context(tc.tile_pool(name="rbp", bufs=4))
        rdp = ctx.enter_context(tc.tile_pool(name="rdp", bufs=4, space="DRAM"))
        stg = ctx.enter_context(tc.tile_pool(name="stg", bufs=4))

        # ---- phase 1: QKV ----
        def rotary_combo(tau, h):
            # per-(q|k, head) rotary over the full block; emitted just
            # before that head's transposes so it overlaps the previous
            # head's (ACT-bound) attention. GpSimd takes every 3rd combo.
            eng = nc.gpsimd if (tau * NH + h) % 3 == 2 else nc.vector
            base = tau * NH * D + h * D
            sl = qk_blk[:, :, base : base + D]
            t2 = t2p.tile([128, 8, D], bf16, tag="t2", name=f"t2_{tau}_{h}")
            eng.tensor_mul(
                out=t2[:, :, 0:40],
                in0=qk_blk[:, :, base + 40 : base + D],
                in1=sin_t[:, :, 0:40],
            )
            eng.tensor_mul(
                out=t2[:, :, 40:D],
                in0=qk_blk[:, :, base : base + 40],
                in1=sin_t[:, :, 40:D],
            )
            eng.tensor_mul(out=sl, in0=sl, in1=cos_t)
            eng.tensor_add(out=sl, in0=sl, in1=t2)

        for m in range(8):
            ps_a = bps.tile([128, 1024], f32, tag="big", name=f"qkva_{m}")
            ps_b = bps.tile([128, 1024], f32, tag="big", name=f"qkvb_{m}")
            for j in range(4):
                ps = ps_a if j < 2 else ps_b
                dst = ps[:, (j % 2) * 512 : (j % 2) * 512 + 480]
                for k in range(10):
                    nc.tensor.matmul(
                        out=dst,
                        lhsT=xt_sb[:, k, m * 128 : (m + 1) * 128],
                        rhs=wt_sb[:, k, j * 480 : (j + 1) * 480],
                        start=(k == 0),
                        stop=(k == 9),
                    )
            # bias adds: q,k -> qk_blk; v -> v_sb
            nc.vector.tensor_add(
                out=qk_blk[:, m, 0:480], in0=ps_a[:, 0:480], in1=bias_bc[:, 0:480]
            )
            nc.vector.tensor_add(
                out=qk_blk[:, m, 480:960],
                in0=ps_a[:, 512:992],
                in1=bias_bc[:, 480:960],
            )
            nc.vector.tensor_add(
                out=qk_blk[:, m, 960:1280],
                in0=ps_b[:, 0:320],
                in1=bias_bc[:, 960:1280],
            )
            nc.vector.tensor_add(
                out=v_sb[:, m, 0:2, 0:D],
                in0=ps_b[:, 320:480].rearrange("p (h d) -> p h d", h=2),
                in1=bias_bc[:, 1280:1440].rearrange("p (h d) -> p h d", h=2),
            )
            nc.vector.tensor_add(
                out=v_sb[:, m, 2:NH, 0:D],
                in0=ps_b[:, 512:992].rearrange("p (h d) -> p h d", h=6),
                in1=bias_bc[:, 1440:1920].rearrange("p (h d) -> p h d", h=6),
            )


        # ---- phases 2+3 interleaved per head: transposes then attention ----
        def transpose_head(h):
            for tau in (1, 0):  # k first so scores unblock sooner
                for g in range(2):
                    base = tau * NH * D + h * D
                    tp = sps.tile(
                        [97, 512], bf16, tag="small", name=f"tr_{g}_{tau}_{h}"
                    )
                    for j in range(4):
                        m = g * 4 + j
                        nc.tensor.matmul(
                            out=tp[0:D, j * 128 : (j + 1) * 128],
                            lhsT=qk_blk[:, m, base : base + D],
                            rhs=ident,
                            is_transpose=True,
                            start=(j == 0),
                            stop=(j == 3),
                        )
                    dst = (qT_sb if tau == 0 else kT_sb)[
                        :, h, g * 512 : (g + 1) * 512
                    ]
                    nc.vector.tensor_copy(out=dst, in_=tp[0:D, :])

        for h in range(NH):
            rotary_combo(1, h)
            rotary_combo(0, h)
            transpose_head(h)
            for sh in range(2):
                qs = qT_sb[:, h, sh * 512 : (sh + 1) * 512]
                pv = sps.tile([97, 512], f32, tag="small", name=f"pv_{h}_{sh}")
                eps = []
                for grp, ts in enumerate(((0, 1), (2, 3), (4, 5), (6, 7))):
                    sc = bps.tile([128, 1024], f32, tag="big", name=f"sc_{h}_{sh}_{grp}")
                    for i, t in enumerate(ts):
                        nc.tensor.matmul(
                            out=sc[:, i * 512 : (i + 1) * 512],
                            lhsT=kT_sb[:, h, t * 128 : (t + 1) * 128],
                            rhs=qs,
                            start=True,
                            stop=True,
                        )
                    w = len(ts) * 512
                    ep = expp.tile([128, 1024], bf16, tag="exp", name=f"ep_{h}_{sh}_{grp}")
                    nc.scalar.activation(
                        out=ep[:, 0:w], in_=sc[:, 0:w], func=EXP, scale=SCALING
                    )
                    eps.append(ep)
                    for i, t in enumerate(ts):
                        nc.tensor.matmul(
                            out=pv,
                            lhsT=v_sb[:, t, h, :],
                            rhs=ep[:, i * 512 : (i + 1) * 512],
                            start=(t == 0),
                            stop=(t == 7),
                        )
                # softmax normalization
                if use_div:
                    # raw den -> broadcast -> DVE divide
                    re_t = rep.tile([1, 512], bf16, tag="re")
                    nc.scalar.copy(out=re_t, in_=pv[96:97, :])
                else:
                    # 1/den via Ln -> Exp(-x), then broadcast + DVE mul
                    ln_t = lnp.tile([1, 512], f32, tag="ln")
                    nc.scalar.activation(out=ln_t, in_=pv[96:97, :], func=LN)
                    re_t = rep.tile([1, 512], bf16, tag="re")
                    nc.scalar.activation(out=re_t, in_=ln_t, func=EXP, scale=-1.0)
                rd_t = rdp.tile([1, 512], bf16, tag="rd")
                nc.sync.dma_start(out=rd_t, in_=re_t)
                rb_t = rbp.tile([D, 512], bf16, tag="rb")
                nc.sync.dma_start(out=rb_t, in_=rd_t[0:1, :].to_broadcast((D, 512)))
                ot_dst = ot_sb[h][:, sh * 512 : (sh + 1) * 512]
                if use_div:
                    nc.vector.tensor_tensor(
                        out=ot_dst,
                        in0=pv[0:D, :],
                        in1=rb_t,
                        op=mybir.AluOpType.divide,
                    )
                else:
                    nc.vector.tensor_mul(out=ot_dst, in0=pv[0:D, :], in1=rb_t)
            # repack this head into the 128-deep contraction layout
            r0 = h * D
            a = r0
            while a < r0 + D:
                j, p0 = a // 128, a % 128
                b_ = min(r0 + D, (j + 1) * 128)
                nc.sync.dma_start(
                    out=ao[p0 : p0 + (b_ - a), j, :],
                    in_=ot_sb[h][a - r0 : b_ - r0, :],
                )
                a = b_

        # ---- phase 5: output projection ----
        for sh in range(2):
            for o in range(10):
                pp = bps.tile([128, 1024], f32, tag="big", name=f"pr_{sh}_{o}")
                for j in range(5):
                    nc.tensor.matmul(
                        out=pp[:, 0:512],
                        lhsT=pw_sb[:, j, o * 128 : (o + 1) * 128],
                        rhs=ao[:, j, sh * 512 : (sh + 1) * 512],
                        start=(j == 0),
                        stop=(j == 4),
                    )
                st = stg.tile([128, 512], f32, tag="st")
                if (sh * 10 + o) % 2 == 0:
                    nc.scalar.copy(out=st, in_=pp[:, 0:512])
                else:
                    nc.vector.tensor_copy(out=st, in_=pp[:, 0:512])
                nc.sync.dma_start(
                    out=outp[
                        o * 128 : (o + 1) * 128, sh * 512 : (sh + 1) * 512
                    ],
                    in_=st,
                )

    nc.compile()
    return nc


def _analyze_mask(mask):
    m = np.asarray(mask).reshape(S, S)
    allowed = []
    mask_add = set()
    for qb in range(NB):
        row = []
        for kb in range(NB):
            t = m[qb * BS : (qb + 1) * BS, kb * BS : (kb + 1) * BS]
            if np.all(t <= NEG_THRESH):
                continue
            row.append(kb)
            if not np.all(t == 0.0):
                mask_add.add((qb, kb))
        if not row:
            raise NotImplementedError("fully masked query block")
        allowed.append(tuple(row))
    return tuple(allowed), frozenset(mask_add)


def _kernel_diag(X, cos, sin, qkv_w, qkv_b, proj_w, proj_b, trace):
    import ml_dtypes

    from concourse import bass_utils

    bf = ml_dtypes.bfloat16
    use_div = bool(int(os.environ.get("KERNEL_DIV", "0")))
    key = ("diag", use_div)
    if key not in _CACHE:
        _CACHE[key] = _build_diag(use_div)
    nc = _CACHE[key]

    B = 1024
    XT = np.ascontiguousarray(X.T)
    cos = np.asarray(cos, dtype=np.float32)
    sin = np.asarray(sin, dtype=np.float32)
    sinh = np.concatenate([-sin[:, : D // 2], sin[:, D // 2 :]], axis=1)
    qkv_w = np.asarray(qkv_w, dtype=np.float32)
    qkv_b = np.asarray(qkv_b, dtype=np.float32)
    proj_w = np.asarray(proj_w, dtype=np.float32)
    proj_b = np.asarray(proj_b, dtype=np.float32)
    vones = np.concatenate([np.zeros(16, bf), np.ones(1, bf)])[None, :]

    in_maps = []
    for c in range(NCORES):
        b, half = c // 2, c % 2
        j0 = half * 8 * D
        hs = slice(j0, j0 + 8 * D)
        Wc = np.concatenate(
            [qkv_w[0:HID][hs], qkv_w[HID : 2 * HID][hs], qkv_w[2 * HID :][hs]],
            axis=0,
        )
        bc = np.concatenate(
            [qkv_b[0:HID][hs], qkv_b[HID : 2 * HID][hs], qkv_b[2 * HID :][hs]]
        )
        in_maps.append(
            {
                "xt": np.ascontiguousarray(XT[:, b * B : (b + 1) * B]).astype(bf),
                "wt": np.ascontiguousarray(Wc.T).astype(bf),
                "bqkv": np.ascontiguousarray(bc[None, :]).astype(bf),
                "cosd": np.ascontiguousarray(cos[b * B : (b + 1) * B]).astype(bf),
                "sind": np.ascontiguousarray(sinh[b * B : (b + 1) * B]).astype(bf),
                "pw": np.ascontiguousarray(proj_w[:, hs].T).astype(bf),
                "vones": vones,
            }
        )

    res = bass_utils.run_bass_kernel_spmd(
        nc, in_maps, core_ids=list(range(NCORES)), trace=trace
    )
    global LAST_RESULT
    LAST_RESULT = res

    out = np.empty((S, HID), dtype=np.float32)
    for b in range(NB):
        acc = res.results[2 * b]["outp"].astype(np.float64) + res.results[
            2 * b + 1
        ]["outp"].astype(np.float64)
        out[b * B : (b + 1) * B] = (
            acc.T + proj_b.astype(np.float64)[None, :]
        ).astype(np.float32)
    return out


def kernel(
    hidden_states, attention_mask, cos, sin, qkv_w, qkv_b, proj_w, proj_b
):
    from concourse import bass_utils

    dt_big = os.environ.get("KERNEL_DT", "float32r")
    out_dt = os.environ.get("KERNEL_OUT_DT", "float32")
    qkv_dt = os.environ.get("KERNEL_QKV_DT", "") or None
    trace = bool(int(os.environ.get("KERNEL_TRACE", "0")))

    X = np.ascontiguousarray(np.asarray(hidden_states, dtype=np.float32))
    allowed, mask_add = _analyze_mask(attention_mask)

    use_diag = (
        allowed == tuple((b,) for b in range(NB))
        and not mask_add
        and not int(os.environ.get("KERNEL_LEGACY", "0"))
    )
    if use_diag:
        return _kernel_diag(X, cos, sin, qkv_w, qkv_b, proj_w, proj_b, trace)

    key = (dt_big, out_dt, qkv_dt, allowed, mask_add)
    if key not in _CACHE:
        _CACHE[key] = _build(
            dt_big, allowed, mask_add, out_dt_name=out_dt, qkv_dt_name=qkv_dt
        )
    nc = _CACHE[key]

    XT = np.ascontiguousarray(X.T)
    cos = np.ascontiguousarray(np.asarray(cos, dtype=np.float32))
    sin = np.asarray(sin, dtype=np.float32)
    sinh = np.ascontiguousarray(
        np.concatenate([-sin[:, : D // 2], sin[:, D // 2 :]], axis=1)
    )
    qkv_w = np.asarray(qkv_w, dtype=np.float32)
    qkv_b = np.asarray(qkv_b, dtype=np.float32)
    proj_w = np.asarray(proj_w, dtype=np.float32)
    proj_b = np.asarray(proj_b, dtype=np.float32)

    in_maps = []
    for c in range(NCORES):
        j0 = c * NHL * D
        sl = slice(j0, j0 + NHL * D)
        Wc = np.concatenate(
            [qkv_w[sl], qkv_w[HID:][sl], qkv_w[2 * HID :][sl]], axis=0
        )
        np_qkv = np.float32
        if qkv_dt == "bfloat16":
            import ml_dtypes

            np_qkv = ml_dtypes.bfloat16
        m = {
            "xt": XT.astype(np_qkv),
            "vpad": np.ascontiguousarray(
                np.tile(
                    np.concatenate(
                        [np.zeros(16, np.float32), np.ones(1, np.float32)]
                    ),
                    (2, 1),
                )
            ),
            "wt": np.ascontiguousarray(Wc.T).astype(np_qkv),
            "bqkv": np.ascontiguousarray(
                np.concatenate([qkv_b[sl], qkv_b[HID:][sl], qkv_b[2 * HID :][sl]])[
                    None, :
                ]
            ),
            "cosd": cos,
            "sind": sinh,
            "pw": np.ascontiguousarray(
                np.stack(
                    [
                        proj_w[:, j0 : j0 + D].T,
                        proj_w[:, j0 + D : j0 + 2 * D].T,
                    ]
                )
            ),
        }
        if mask_add:
            m["maskt"] = np.ascontiguousarray(
                (np.asarray(attention_mask).reshape(S, S).T / SCALING).astype(
                    np.float32
                )
            )
        in_maps.append(m)

    res = bass_utils.run_bass_kernel_spmd(
        nc, in_maps, core_ids=list(range(NCORES)), trace=trace
    )
    global LAST_RESULT
    LAST_RESULT = res

    acc = np.zeros((HID, S), dtype=np.float64)
    for c in range(NCORES):
        acc += res.results[c]["outp"]
    out = acc.T + proj_b.astype(np.float64)[None, :]
    return out.astype(np.float32)


LAST_RESULT = None

